# revision 1
# baseline (speedup 1.0000x reference)
"""DrBCNet GNN message-passing kernel for 8 Trainium2 NeuronCores.

Strategy (dst-sharded graph parallel):
  - Nodes are globally degree-sorted and dealt round-robin to the 8 cores
    (3750 each); each core owns its dst nodes, GRU/norm/decoder are node-local.
  - Per layer: the bf16 h-table (all-gathered each layer) stays SBUF-resident;
    TensorE accumulates aggT[feat, dst] = sum_blocks table_block.T @ A_block
    in PSUM, streaming the per-core bf16 adjacency A [table_rows, npc] from
    HBM as fp8 (exact 0/1/2 counts; memory-roofline bound). The bank range is
    processed in two halves so the first half's GRU/l2norm (ACT/DVE) overlaps
    the second half's matmuls. PE transposes h back to row layout which is
    AllGather'd (bf16) into the next layer's table.
"""

import functools
import os

import numpy as np

CORES = 8
H = 128
L = 5
BANK = 512  # fp32 PSUM bank width
NORM_EPS_SQ = 1e-24


# ---------------------------------------------------------------- host planning
def _plan(edge_src, edge_dst, n_nodes):
    npc = n_nodes // CORES
    npc_pad = ((npc + 127) // 128) * 128
    ntiles = npc_pad // 128

    # global degree sort, dealt round-robin to cores: core = rank % 8,
    # pos = rank // 8. Interleaved sorted sequences keep the per-position
    # cumulative degree nearly identical across cores (bounded drift), so
    # shared per-chunk PSUM column windows stay narrow.
    deg = np.bincount(edge_dst, minlength=n_nodes)
    gorder = np.argsort(-deg, kind="stable")  # rank -> node
    gpos = np.empty(n_nodes, np.int64)
    gpos[gorder] = np.arange(n_nodes)
    owner = gpos % CORES
    pos = gpos // CORES
    order_per_core = [gorder[r::CORES] for r in range(CORES)]  # pos -> node id
    core_edges = [np.nonzero(owner[edge_dst] == r)[0] for r in range(CORES)]

    tpos = owner * npc_pad + pos  # node -> table row
    n_banks = (npc + BANK - 1) // BANK
    import ml_dtypes

    n_tbl = CORES * npc_pad
    A = []
    for r in range(CORES):
        eidx = core_edges[r]
        Af = np.zeros((n_tbl, npc), np.float32)
        np.add.at(Af, (tpos[edge_src[eidx]], pos[edge_dst[eidx]]), 1.0)
        A.append(Af.astype(ml_dtypes.float8_e4m3fn))
    return dict(
        npc=npc,
        npc_pad=npc_pad,
        ntiles=ntiles,
        n_banks=n_banks,
        n_tbl=n_tbl,
        A=A,
        order_per_core=order_per_core,
        tpos=tpos,
    )


# ---------------------------------------------------------------- bass program
def _build(meta):
    import concourse.bacc as bacc
    import concourse.mybir as mybir
    import concourse.tile as tile
    from concourse.masks import make_identity

    npc = meta["npc"]
    npc_pad = meta["npc_pad"]
    ntiles = meta["ntiles"]
    n_banks = meta["n_banks"]
    n_tbl = meta["n_tbl"]
    n_blk = n_tbl // 128
    f32 = mybir.dt.float32
    bf16 = mybir.dt.bfloat16
    AF = mybir.ActivationFunctionType
    OP = mybir.AluOpType

    nc = bacc.Bacc(
        "TRN2", target_bir_lowering=False, debug=False, num_devices=CORES
    )

    # I/O
    xT_d = nc.dram_tensor("xT", [3, npc], f32, kind="ExternalInput")
    A_d = nc.dram_tensor("A", [n_tbl, npc], mybir.dt.float8e4, kind="ExternalInput")
    w1T_d = nc.dram_tensor("w1T", [3, 128], f32, kind="ExternalInput")
    b1_d = nc.dram_tensor("b1", [128, 1], f32, kind="ExternalInput")
    wihT_d = nc.dram_tensor("wihT", [128, 3 * H], f32, kind="ExternalInput")
    whhT_d = nc.dram_tensor("whhT", [128, 3 * H], f32, kind="ExternalInput")
    bih_d = nc.dram_tensor("bih", [1, 3 * H], f32, kind="ExternalInput")
    bhh_d = nc.dram_tensor("bhh", [1, 3 * H], f32, kind="ExternalInput")
    w2T_d = nc.dram_tensor("w2T", [128, 128], f32, kind="ExternalInput")
    b2_d = nc.dram_tensor("b2", [1, 128], f32, kind="ExternalInput")
    out_d = nc.dram_tensor("out", [npc_pad, 128], f32, kind="ExternalOutput")

    ag_in = [nc.dram_tensor(f"agin{l}", [npc_pad, 128], bf16) for l in range(L)]
    tables = [
        nc.dram_tensor(f"table{l}", [CORES * npc_pad, 128], bf16, addr_space="Shared")
        for l in range(L)
    ]
    groups = [list(range(CORES))]

    banks = [(b * BANK, min(BANK, npc - b * BANK)) for b in range(n_banks)]

    with tile.TileContext(nc) as tc:
        import contextlib

        stack = contextlib.ExitStack()
        per = stack.enter_context(tc.tile_pool(name="per", bufs=1))

        def _T(tc, shape, dtype, name=None):
            return per.tile(shape, dtype, name=name, tag=name)

        # persistent tiles
        table_sb = _T(tc, [128, n_blk, 128], bf16, name="table_sb")
        xT_sb = _T(tc, [3, npc], f32, name="xT_sb")
        hT = _T(tc, [128, npc], f32, name="hT")
        hmaxT = _T(tc, [128, npc], f32, name="hmaxT")
        aggT = _T(tc, [128, npc], f32, name="aggT")
        w1T_sb = _T(tc, [3, 128], f32, name="w1T_sb")
        b1_sb = _T(tc, [128, 1], f32, name="b1_sb")
        wihT_sb = _T(tc, [128, 3 * H], f32, name="wihT_sb")
        whhT_sb = _T(tc, [128, 3 * H], f32, name="whhT_sb")
        bih_sb = _T(tc, [1, 3 * H], f32, name="bih_sb")
        bhh_sb = _T(tc, [1, 3 * H], f32, name="bhh_sb")
        w2T_sb = _T(tc, [128, 128], f32, name="w2T_sb")
        b2_sb = _T(tc, [1, 128], f32, name="b2_sb")
        ones_col = _T(tc, [128, 1], f32, name="ones_col")
        ones_row = _T(tc, [1, BANK], f32, name="ones_row")
        onesk1 = _T(tc, [1, 128], f32, name="onesk1")
        ident = _T(tc, [128, 128], f32, name="ident")
        eps_sb = _T(tc, [1, 1], f32, name="eps_sb")

        xpool = stack.enter_context(tc.tile_pool(name="xpool", bufs=2))
        apool = stack.enter_context(tc.tile_pool(name="apool", bufs=2))
        gpool = stack.enter_context(tc.tile_pool(name="gpool", bufs=2))
        tpool = stack.enter_context(tc.tile_pool(name="tpool", bufs=1))
        ps = stack.enter_context(tc.tile_pool(name="ps", bufs=8, space="PSUM"))

        # input loads
        nc.sync.dma_start(out=xT_sb[:], in_=xT_d[:])
        nc.sync.dma_start(out=w1T_sb[:], in_=w1T_d[:])
        nc.sync.dma_start(out=b1_sb[:], in_=b1_d[:])
        nc.sync.dma_start(out=wihT_sb[:], in_=wihT_d[:])
        nc.sync.dma_start(out=whhT_sb[:], in_=whhT_d[:])
        nc.sync.dma_start(out=bih_sb[:], in_=bih_d[:])
        nc.sync.dma_start(out=bhh_sb[:], in_=bhh_d[:])
        nc.sync.dma_start(out=w2T_sb[:], in_=w2T_d[:])
        nc.sync.dma_start(out=b2_sb[:], in_=b2_d[:])
        nc.vector.memset(eps_sb[:], NORM_EPS_SQ)
        nc.vector.memset(ones_col[:], 1.0)
        nc.vector.memset(ones_row[:], 1.0)
        nc.vector.memset(onesk1[:], 1.0)
        make_identity(nc, ident[:])

        def norm_strip(b, s0, w):
            """hT[:, s0:s0+w] /= (sqrt(sum_f hT^2) + eps), per node column."""
            sq = tpool.tile([128, BANK], f32, tag="sq")
            nc.vector.tensor_tensor(
                out=sq[:, :w], in0=hT[:, s0 : s0 + w], in1=hT[:, s0 : s0 + w],
                op=OP.mult,
            )
            ns_ps = ps.tile([1, BANK], f32, tag="ps", name=f"ns{b}")
            nc.tensor.matmul(
                out=ns_ps[:1, :w], lhsT=ones_col[:], rhs=sq[:, :w],
                start=True, stop=True,
            )
            srt = tpool.tile([1, BANK], f32, tag="srt")
            nc.scalar.activation(
                out=srt[:1, :w], in_=ns_ps[:1, :w], func=AF.Sqrt, bias=eps_sb[:1, :1]
            )
            inv_t = tpool.tile([1, BANK], f32, tag="inv_t")
            nc.vector.reciprocal(out=inv_t[:1, :w], in_=srt[:1, :w])
            bc_ps = ps.tile([128, BANK], f32, tag="ps", name=f"bc{b}")
            nc.tensor.matmul(
                out=bc_ps[:, :w], lhsT=onesk1[:1, :], rhs=inv_t[:1, :w],
                start=True, stop=True,
            )
            nc.vector.tensor_tensor(
                out=hT[:, s0 : s0 + w], in0=hT[:, s0 : s0 + w], in1=bc_ps[:, :w],
                op=OP.mult,
            )

        def store_and_allgather(l):
            rows = xpool.tile([128, ntiles, 128], bf16, tag="xbuf", name=f"rows{l}")
            if npc - (ntiles - 1) * 128 < 128:
                nc.vector.memset(rows[:, ntiles - 1, :], 0.0)
            for t in range(ntiles):
                wt = min(128, npc - t * 128)
                if wt <= 0:
                    break
                tp_ps = ps.tile([128, 128], f32, tag="ps", name=f"tp{l}_{t}")
                nc.tensor.transpose(
                    out=tp_ps[:wt, :], in_=hT[:, t * 128 : t * 128 + wt],
                    identity=ident[:],
                )
                nc.scalar.activation(
                    out=rows[:wt, t, :], in_=tp_ps[:wt, :], func=AF.Copy
                )
            dst = ag_in[l].ap().rearrange("(c p) f -> p c f", p=128)
            nc.sync.dma_start(out=dst, in_=rows[:])
            nc.gpsimd.collective_compute(
                "AllGather",
                OP.bypass,
                replica_groups=groups,
                ins=[ag_in[l][:]],
                outs=[tables[l][:]],
            )

        # ---------------- encoder: hT = l2norm(relu(W1 @ x + b1)), hmax = hT
        for b, (s0, w) in enumerate(banks):
            h0_ps = ps.tile([128, BANK], f32, tag="ps", name=f"enc{b}")
            nc.tensor.matmul(
                out=h0_ps[:, :w], lhsT=w1T_sb[:], rhs=xT_sb[:, s0 : s0 + w],
                start=True, stop=True,
            )
            nc.scalar.activation(
                out=hT[:, s0 : s0 + w], in_=h0_ps[:, :w], func=AF.Relu,
                bias=b1_sb[:, :1],
            )
            norm_strip(b, s0, w)
            nc.vector.tensor_copy(out=hmaxT[:, s0 : s0 + w], in_=hT[:, s0 : s0 + w])
        store_and_allgather(0)

        # ---------------- message-passing layers
        for l in range(L):
            # full bf16 table -> SBUF as [128 rows-in-block, block, feat]
            nc.sync.dma_start(
                out=table_sb[:],
                in_=tables[l].ap().rearrange("(u p) f -> p u f", p=128),
            )
            nh = (n_banks + 1) // 2
            halves = [list(enumerate(banks))[:nh], list(enumerate(banks))[nh:]]
            for hi, hbanks in enumerate(halves):
                if not hbanks:
                    continue
                c0 = hbanks[0][1][0]
                c1 = hbanks[-1][1][0] + hbanks[-1][1][1]
                agg_ps = {}
                for b, (s0, w) in hbanks:
                    agg_ps[b] = ps.tile(
                        [128, BANK], f32, tag="ps", name=f"agg{l}_{b}"
                    )
                # batch G src-blocks per A DMA (~1 MB transfers -> full DMA bw)
                G = 4 if n_blk % 4 == 0 else (2 if n_blk % 2 == 0 else 1)
                for ug0 in range(0, n_blk, G):
                    a_sb = apool.tile(
                        [128, G, c1 - c0], mybir.dt.float8e4, tag="a_sb",
                        name=f"a{l}_{hi}_{ug0}",
                    )
                    nc.sync.dma_start(
                        out=a_sb[:],
                        in_=A_d[ug0 * 128 : (ug0 + G) * 128, c0:c1].rearrange(
                            "(g p) c -> p g c", p=128
                        ),
                    )
                    for g in range(G):
                        ug = ug0 + g
                        for b, (s0, w) in hbanks:
                            nc.tensor.matmul(
                                out=agg_ps[b][:, :w],
                                lhsT=table_sb[:, ug, :],
                                rhs=a_sb[:, g, s0 - c0 : s0 - c0 + w],
                                start=(ug == 0),
                                stop=(ug == n_blk - 1),
                            )
                for b, (s0, w) in hbanks:
                    apb = agg_ps[b]
                    # evacuate aggT strip, then GRU for this strip
                    nc.scalar.activation(
                        out=aggT[:, s0 : s0 + w], in_=apb[:, :w], func=AF.Copy
                    )

                    gis, ghs = [], []
                    for g in range(3):
                        gi_ps = ps.tile([128, BANK], f32, tag="ps", name=f"gi{l}{b}{g}")
                        nc.tensor.matmul(
                            out=gi_ps[:, :w], lhsT=bih_sb[:1, g * H : (g + 1) * H],
                            rhs=ones_row[:1, :w], start=True, stop=False,
                        )
                        nc.tensor.matmul(
                            out=gi_ps[:, :w], lhsT=wihT_sb[:, g * H : (g + 1) * H],
                            rhs=aggT[:, s0 : s0 + w], start=False, stop=True,
                        )
                        gi = gpool.tile([128, BANK], f32, tag=f"gi{g}")
                        nc.scalar.activation(out=gi[:, :w], in_=gi_ps[:, :w], func=AF.Copy)
                        gis.append(gi)
                        gh_ps = ps.tile([128, BANK], f32, tag="ps", name=f"gh{l}{b}{g}")
                        nc.tensor.matmul(
                            out=gh_ps[:, :w], lhsT=bhh_sb[:1, g * H : (g + 1) * H],
                            rhs=ones_row[:1, :w], start=True, stop=False,
                        )
                        nc.tensor.matmul(
                            out=gh_ps[:, :w], lhsT=whhT_sb[:, g * H : (g + 1) * H],
                            rhs=hT[:, s0 : s0 + w], start=False, stop=True,
                        )
                        gh = gpool.tile([128, BANK], f32, tag=f"gh{g}")
                        nc.scalar.activation(out=gh[:, :w], in_=gh_ps[:, :w], func=AF.Copy)
                        ghs.append(gh)

                    # r = sig(i_r + h_r); z = sig(i_z + h_z); n = tanh(i_n + r*h_n)
                    r_t = tpool.tile([128, BANK], f32, tag="r_t")
                    nc.vector.tensor_tensor(
                        out=r_t[:, :w], in0=gis[0][:, :w], in1=ghs[0][:, :w], op=OP.add
                    )
                    nc.scalar.activation(out=r_t[:, :w], in_=r_t[:, :w], func=AF.Sigmoid)
                    z_t = tpool.tile([128, BANK], f32, tag="z_t")
                    nc.vector.tensor_tensor(
                        out=z_t[:, :w], in0=gis[1][:, :w], in1=ghs[1][:, :w], op=OP.add
                    )
                    nc.scalar.activation(out=z_t[:, :w], in_=z_t[:, :w], func=AF.Sigmoid)
                    n_t = tpool.tile([128, BANK], f32, tag="n_t")
                    nc.vector.tensor_tensor(
                        out=n_t[:, :w], in0=r_t[:, :w], in1=ghs[2][:, :w], op=OP.mult
                    )
                    nc.vector.tensor_tensor(
                        out=n_t[:, :w], in0=n_t[:, :w], in1=gis[2][:, :w], op=OP.add
                    )
                    nc.scalar.activation(out=n_t[:, :w], in_=n_t[:, :w], func=AF.Tanh)
                    # h' = n + z * (h - n)
                    d_t = tpool.tile([128, BANK], f32, tag="d_t")
                    nc.vector.tensor_tensor(
                        out=d_t[:, :w], in0=hT[:, s0 : s0 + w], in1=n_t[:, :w],
                        op=OP.subtract,
                    )
                    nc.vector.tensor_tensor(
                        out=d_t[:, :w], in0=d_t[:, :w], in1=z_t[:, :w], op=OP.mult
                    )
                    nc.vector.tensor_tensor(
                        out=hT[:, s0 : s0 + w], in0=d_t[:, :w], in1=n_t[:, :w], op=OP.add
                    )
                    norm_strip(b, s0, w)
                    nc.vector.tensor_tensor(
                        out=hmaxT[:, s0 : s0 + w], in0=hmaxT[:, s0 : s0 + w],
                        in1=hT[:, s0 : s0 + w], op=OP.max,
                    )
            if l < L - 1:
                store_and_allgather(l + 1)

        # ---------------- decoder: out = hmax @ W2.T + b2 (row layout)
        for t in range(ntiles):
            wt = min(128, npc - t * 128)
            o_ps = ps.tile([128, 128], f32, tag="ps", name=f"dec{t}")
            nc.tensor.matmul(
                out=o_ps[:wt, :], lhsT=onesk1[:1, :wt], rhs=b2_sb[:1, :],
                start=True, stop=False,
            )
            nc.tensor.matmul(
                out=o_ps[:wt, :], lhsT=hmaxT[:, t * 128 : t * 128 + wt],
                rhs=w2T_sb[:], start=False, stop=True,
            )
            orow = tpool.tile([128, 128], f32, tag="orow")
            nc.scalar.activation(out=orow[:wt, :], in_=o_ps[:wt, :], func=AF.Copy)
            nc.sync.dma_start(
                out=out_d[t * 128 : t * 128 + wt, :], in_=orow[:wt, :]
            )
        stack.close()

    nc.compile()
    return nc


# ---------------------------------------------------------------- entry points
def _prep(inputs):
    x = np.asarray(inputs["x"], np.float32)
    edge_src = np.asarray(inputs["edge_src"], np.int64)
    edge_dst = np.asarray(inputs["edge_dst"], np.int64)
    n_nodes = x.shape[0]
    meta = _plan(edge_src, edge_dst, n_nodes)
    npc = meta["npc"]

    W1 = np.asarray(inputs["W1"], np.float32)
    b1 = np.asarray(inputs["b1"], np.float32)
    W_ih = np.asarray(inputs["W_ih"], np.float32)
    b_ih = np.asarray(inputs["b_ih"], np.float32)
    W_hh = np.asarray(inputs["W_hh"], np.float32)
    b_hh = np.asarray(inputs["b_hh"], np.float32)
    W2 = np.asarray(inputs["W2"], np.float32)
    b2 = np.asarray(inputs["b2"], np.float32)

    shared = dict(
        w1T=np.ascontiguousarray(W1.T),
        b1=np.ascontiguousarray(b1[:, None]),
        wihT=np.ascontiguousarray(W_ih.T),
        whhT=np.ascontiguousarray(W_hh.T),
        bih=np.ascontiguousarray(b_ih[None, :]),
        bhh=np.ascontiguousarray(b_hh[None, :]),
        w2T=np.ascontiguousarray(W2.T),
        b2=np.ascontiguousarray(b2[None, :]),
    )
    in_maps = []
    for r in range(CORES):
        xr = x[meta["order_per_core"][r]]
        in_maps.append(
            dict(
                xT=np.ascontiguousarray(xr.T),
                A=meta["A"][r],
                **shared,
            )
        )
    return meta, in_maps


def _assemble(meta, results, n_nodes):
    npc = meta["npc"]
    out = np.empty((n_nodes, 128), np.float32)
    for r in range(CORES):
        out[meta["order_per_core"][r]] = results[r]["out"][:npc]
    return out


@functools.lru_cache(maxsize=1)
def _get_compiled(key):
    # key is a hash of the planning inputs; real data passed via _PENDING
    meta, in_maps = _PENDING[key]
    nc = _build(meta)
    return nc, meta, in_maps


_PENDING = {}


def kernel(**inputs):
    x = np.asarray(inputs["x"])
    n_nodes = x.shape[0]
    meta, in_maps = _prep(inputs)
    key = hash(
        (
            n_nodes,
            np.asarray(inputs["edge_src"]).tobytes(),
            np.asarray(inputs["edge_dst"]).tobytes(),
        )
    )
    _PENDING[key] = (meta, in_maps)
    nc, meta, _ = _get_compiled(key)

    from concourse.bass_utils import run_bass_kernel_spmd

    trace = bool(int(os.environ.get("KERNEL_TRACE", "0")))
    res = run_bass_kernel_spmd(
        nc, in_maps, core_ids=list(range(CORES)), trace=trace
    )
    kernel.last_results = res
    return _assemble(meta, res.results, n_nodes)



# revision 4
# speedup vs baseline: 1.2939x; 1.2939x over previous
"""DrBCNet GNN message-passing kernel for 8 Trainium2 NeuronCores.

Strategy (dst-sharded graph parallel, edge-compacted sparse aggregation):
  - Nodes are globally degree-sorted and dealt round-robin to the 8 cores
    (3750 each); each core owns its dst nodes, GRU/norm/decoder are node-local.
  - Per layer the bf16 h-table [30720, 128] is AllGather'd to every core's
    HBM. Aggregation is edge-compacted: edges are dst-sorted into 128-dst
    windows; per window a gpsimd dma_gather pulls the ~2.6k source rows
    (256 B each) straight out of the HBM table into an edge-major SBUF tile
    [128 edges x nb x 128 feat], and TensorE contracts it against a host-built
    one-hot S block [128 edges x 128 dstcol] (fp8) accumulating aggT in PSUM.
    That replaces the dense 115 MB/layer adjacency stream with ~20 MB of
    gathers + ~10 MB of S per core per layer (memory-roofline bound by DMA
    descriptor rate: ~79k descs/layer/core).
  - Edge counts per window are padded to a common per-window count across
    cores with dummy (row-0, S=0) edges so the SPMD program is uniform.
  - GRU gates (Sigmoid/Tanh) run per 512-column strip; all l2norm Sqrt calls
    are batched at the layer tail so the ACT engine swaps activation tables
    twice per layer instead of twice per strip.
"""

import functools
import os

import numpy as np

CORES = 8
H = 128
L = 5
BANK = 512  # fp32 PSUM bank width
WIN = 128  # dst-window width for sparse aggregation
GMAX = 1024  # max indices per dma_gather call (HW SWDGE ring batch limit)
NORM_EPS_SQ = 1e-24


# ---------------------------------------------------------------- host planning
def _plan(edge_src, edge_dst, n_nodes):
    import ml_dtypes

    npc = n_nodes // CORES
    npc_pad = ((npc + 127) // 128) * 128
    ntiles = npc_pad // 128
    n_win = ntiles

    # global degree sort, dealt round-robin to cores: core = rank % 8,
    # pos = rank // 8. Keeps per-core edge counts (and per-window counts)
    # nearly identical across cores, so the uniform SPMD padding is small.
    deg = np.bincount(edge_dst, minlength=n_nodes)
    gorder = np.argsort(-deg, kind="stable")  # rank -> node
    gpos = np.empty(n_nodes, np.int64)
    gpos[gorder] = np.arange(n_nodes)
    owner = gpos % CORES
    pos = gpos // CORES
    order_per_core = [gorder[r::CORES] for r in range(CORES)]  # pos -> node id
    tpos = owner * npc_pad + pos  # node -> table row

    # per-core dst-sorted edge lists, bucketed into 128-dst windows
    esrc_w = []  # [core][win] -> table-row idx of each edge's src
    ecol_w = []  # [core][win] -> dst column within window
    counts = np.zeros((CORES, n_win), np.int64)
    for r in range(CORES):
        eidx = np.nonzero(owner[edge_dst] == r)[0]
        p = pos[edge_dst[eidx]]
        o = np.argsort(p, kind="stable")
        es = tpos[edge_src[eidx]][o]
        ep = p[o]
        w = ep // WIN
        counts[r] = np.bincount(w, minlength=n_win)
        split = np.cumsum(counts[r])[:-1]
        esrc_w.append(np.split(es, split))
        ecol_w.append(np.split(ep - w * WIN, split))

    # uniform per-window padded counts (multiple of 128)
    pw = ((counts.max(axis=0) + 127) // 128) * 128  # [n_win]
    nb = pw // 128
    offs = np.concatenate([[0], np.cumsum(pw)])  # edge-slot offsets
    tot = int(offs[-1])
    nb_tot = tot // 128

    idx_wrapped = []
    S_wrapped = []
    for r in range(CORES):
        idx_all = np.zeros(tot, np.int16)
        S_flat = np.zeros((tot, WIN), np.float32)
        for w in range(n_win):
            e = counts[r, w]
            sl = slice(int(offs[w]), int(offs[w]) + int(e))
            idx_all[sl] = esrc_w[r][w].astype(np.int16)
            S_flat[np.arange(int(offs[w]), int(offs[w]) + int(e)), ecol_w[r][w]] = 1.0
        # idx i -> partition i%16, col i//16; replicated x8 over partition groups
        idx_wrapped.append(
            np.ascontiguousarray(np.tile(idx_all.reshape(-1, 16).T, (8, 1)))
        )
        # S stored partition-major: [128, nb_tot*128] so each partition's
        # window slice is one contiguous DMA run
        S_wrapped.append(
            np.ascontiguousarray(
                S_flat.reshape(nb_tot, 128, WIN)
                .transpose(1, 0, 2)
                .reshape(128, nb_tot * WIN)
                .astype(ml_dtypes.float8_e4m3fn)
            )
        )

    return dict(
        npc=npc,
        npc_pad=npc_pad,
        ntiles=ntiles,
        n_win=n_win,
        n_tbl=CORES * npc_pad,
        pw=pw.astype(np.int64),
        nb=nb.astype(np.int64),
        offs=offs.astype(np.int64),
        tot=tot,
        nb_tot=nb_tot,
        idx_wrapped=idx_wrapped,
        S_wrapped=S_wrapped,
        order_per_core=order_per_core,
    )


# ---------------------------------------------------------------- bass program
def _build(meta):
    import concourse.bacc as bacc
    import concourse.mybir as mybir
    import concourse.tile as tile
    from concourse.masks import make_identity

    npc = meta["npc"]
    npc_pad = meta["npc_pad"]
    ntiles = meta["ntiles"]
    n_win = meta["n_win"]
    n_tbl = meta["n_tbl"]
    pw = meta["pw"]
    nb = meta["nb"]
    offs = meta["offs"]
    tot = meta["tot"]
    nb_max = int(nb.max())
    f32 = mybir.dt.float32
    bf16 = mybir.dt.bfloat16
    i16 = mybir.dt.int16
    fp8 = mybir.dt.float8e4
    AF = mybir.ActivationFunctionType
    OP = mybir.AluOpType

    n_banks = (npc + BANK - 1) // BANK
    banks = [(b * BANK, min(BANK, npc - b * BANK)) for b in range(n_banks)]

    nc = bacc.Bacc(
        "TRN2", target_bir_lowering=False, debug=False, num_devices=CORES
    )

    # I/O
    xT_d = nc.dram_tensor("xT", [3, npc], f32, kind="ExternalInput")
    idx_d = nc.dram_tensor("idxw", [128, tot // 16], i16, kind="ExternalInput")
    S_d = nc.dram_tensor("S", [128, tot], fp8, kind="ExternalInput")
    w1T_d = nc.dram_tensor("w1T", [3, 128], f32, kind="ExternalInput")
    b1_d = nc.dram_tensor("b1", [128, 1], f32, kind="ExternalInput")
    wihT_d = nc.dram_tensor("wihT", [128, 3 * H], f32, kind="ExternalInput")
    whhT_d = nc.dram_tensor("whhT", [128, 3 * H], f32, kind="ExternalInput")
    bih_d = nc.dram_tensor("bih", [1, 3 * H], f32, kind="ExternalInput")
    bhh_d = nc.dram_tensor("bhh", [1, 3 * H], f32, kind="ExternalInput")
    w2T_d = nc.dram_tensor("w2T", [128, 128], f32, kind="ExternalInput")
    b2_d = nc.dram_tensor("b2", [1, 128], f32, kind="ExternalInput")
    out_d = nc.dram_tensor("out", [npc_pad, 128], f32, kind="ExternalOutput")

    ag_in = [nc.dram_tensor(f"agin{l}", [npc_pad, 128], bf16) for l in range(L)]
    tables = [
        nc.dram_tensor(f"table{l}", [n_tbl, 128], bf16, addr_space="Shared")
        for l in range(L)
    ]
    groups = [list(range(CORES))]

    with tile.TileContext(nc) as tc:
        import contextlib

        stack = contextlib.ExitStack()
        per = stack.enter_context(tc.tile_pool(name="per", bufs=1))

        def _T(tc, shape, dtype, name=None):
            return per.tile(shape, dtype, name=name, tag=name)

        # persistent tiles
        idx_sb = _T(tc, [128, tot // 16], i16, name="idx_sb")
        xT_sb = _T(tc, [3, npc], f32, name="xT_sb")
        hT = _T(tc, [128, npc], f32, name="hT")
        hmaxT = _T(tc, [128, npc], f32, name="hmaxT")
        aggT = _T(tc, [128, npc], f32, name="aggT")
        w1T_sb = _T(tc, [3, 128], f32, name="w1T_sb")
        b1_sb = _T(tc, [128, 1], f32, name="b1_sb")
        wihT_sb = _T(tc, [128, 3 * H], f32, name="wihT_sb")
        whhT_sb = _T(tc, [128, 3 * H], f32, name="whhT_sb")
        bih_sb = _T(tc, [1, 3 * H], f32, name="bih_sb")
        bhh_sb = _T(tc, [1, 3 * H], f32, name="bhh_sb")
        w2T_sb = _T(tc, [128, 128], f32, name="w2T_sb")
        b2_sb = _T(tc, [1, 128], f32, name="b2_sb")
        ones_col = _T(tc, [128, 1], f32, name="ones_col")
        ones_row = _T(tc, [1, BANK], f32, name="ones_row")
        onesk1 = _T(tc, [1, 128], f32, name="onesk1")
        ident = _T(tc, [128, 128], f32, name="ident")
        eps_sb = _T(tc, [1, 1], f32, name="eps_sb")

        xpool = stack.enter_context(tc.tile_pool(name="xpool", bufs=2))
        gbpool = stack.enter_context(tc.tile_pool(name="gbpool", bufs=2))
        sbpool = stack.enter_context(tc.tile_pool(name="sbpool", bufs=2))
        gpool = stack.enter_context(tc.tile_pool(name="gpool", bufs=2))
        tpool = stack.enter_context(tc.tile_pool(name="tpool", bufs=1))
        ps = stack.enter_context(tc.tile_pool(name="ps", bufs=8, space="PSUM"))

        # input loads
        nc.sync.dma_start(out=idx_sb[:], in_=idx_d[:])
        nc.sync.dma_start(out=xT_sb[:], in_=xT_d[:])
        nc.sync.dma_start(out=w1T_sb[:], in_=w1T_d[:])
        nc.sync.dma_start(out=b1_sb[:], in_=b1_d[:])
        nc.sync.dma_start(out=wihT_sb[:], in_=wihT_d[:])
        nc.sync.dma_start(out=whhT_sb[:], in_=whhT_d[:])
        nc.sync.dma_start(out=bih_sb[:], in_=bih_d[:])
        nc.sync.dma_start(out=bhh_sb[:], in_=bhh_d[:])
        nc.sync.dma_start(out=w2T_sb[:], in_=w2T_d[:])
        nc.sync.dma_start(out=b2_sb[:], in_=b2_d[:])
        nc.vector.memset(eps_sb[:], NORM_EPS_SQ)
        nc.vector.memset(ones_col[:], 1.0)
        nc.vector.memset(ones_row[:], 1.0)
        nc.vector.memset(onesk1[:], 1.0)
        make_identity(nc, ident[:])

        def norm_strip(b, s0, w):
            """hT[:, s0:s0+w] /= (sqrt(sum_f hT^2) + eps), per node column."""
            sq = tpool.tile([128, BANK], f32, tag="sq")
            nc.vector.tensor_tensor(
                out=sq[:, :w], in0=hT[:, s0 : s0 + w], in1=hT[:, s0 : s0 + w],
                op=OP.mult,
            )
            ns_ps = ps.tile([1, BANK], f32, tag="ps", name=f"ns{b}")
            nc.tensor.matmul(
                out=ns_ps[:1, :w], lhsT=ones_col[:], rhs=sq[:, :w],
                start=True, stop=True,
            )
            srt = tpool.tile([1, BANK], f32, tag="srt")
            nc.scalar.activation(
                out=srt[:1, :w], in_=ns_ps[:1, :w], func=AF.Sqrt, bias=eps_sb[:1, :1]
            )
            inv_t = tpool.tile([1, BANK], f32, tag="inv_t")
            nc.vector.reciprocal(out=inv_t[:1, :w], in_=srt[:1, :w])
            bc_ps = ps.tile([128, BANK], f32, tag="ps", name=f"bc{b}")
            nc.tensor.matmul(
                out=bc_ps[:, :w], lhsT=onesk1[:1, :], rhs=inv_t[:1, :w],
                start=True, stop=True,
            )
            nc.vector.tensor_tensor(
                out=hT[:, s0 : s0 + w], in0=hT[:, s0 : s0 + w], in1=bc_ps[:, :w],
                op=OP.mult,
            )

        def store_and_allgather(l):
            rows = xpool.tile([128, ntiles, 128], bf16, tag="xbuf", name=f"rows{l}")
            if npc - (ntiles - 1) * 128 < 128:
                nc.vector.memset(rows[:, ntiles - 1, :], 0.0)
            for t in range(ntiles):
                wt = min(128, npc - t * 128)
                if wt <= 0:
                    break
                tp_ps = ps.tile([128, 128], f32, tag="ps", name=f"tp{l}_{t}")
                nc.tensor.transpose(
                    out=tp_ps[:wt, :], in_=hT[:, t * 128 : t * 128 + wt],
                    identity=ident[:],
                )
                nc.scalar.activation(
                    out=rows[:wt, t, :], in_=tp_ps[:wt, :], func=AF.Copy
                )
            dst = ag_in[l].ap().rearrange("(c p) f -> p c f", p=128)
            nc.sync.dma_start(out=dst, in_=rows[:])
            nc.gpsimd.collective_compute(
                "AllGather",
                OP.bypass,
                replica_groups=groups,
                ins=[ag_in[l][:]],
                outs=[tables[l][:]],
            )

        # ---------------- encoder: hT = l2norm(relu(W1 @ x + b1)), hmax = hT
        for b, (s0, w) in enumerate(banks):
            h0_ps = ps.tile([128, BANK], f32, tag="ps", name=f"enc{b}")
            nc.tensor.matmul(
                out=h0_ps[:, :w], lhsT=w1T_sb[:], rhs=xT_sb[:, s0 : s0 + w],
                start=True, stop=True,
            )
            nc.scalar.activation(
                out=hT[:, s0 : s0 + w], in_=h0_ps[:, :w], func=AF.Relu,
                bias=b1_sb[:, :1],
            )
            norm_strip(b, s0, w)
            nc.vector.tensor_copy(out=hmaxT[:, s0 : s0 + w], in_=hT[:, s0 : s0 + w])
        store_and_allgather(0)

        # ---------------- message-passing layers
        for l in range(L):
            # sparse aggregation: per 128-dst window, gather source rows from
            # the HBM table (edge-major) and contract with one-hot S in PSUM
            for w in range(n_win):
                wcols = min(WIN, npc - w * WIN)
                if pw[w] == 0:
                    nc.vector.memset(aggT[:, w * WIN : w * WIN + wcols], 0.0)
                    continue
                nbw = int(nb[w])
                o = int(offs[w])
                gb = gbpool.tile(
                    [128, nb_max, 128], bf16, tag="gb", name=f"gb{l}_{w}"
                )
                sb = sbpool.tile(
                    [128, nb_max * 128], fp8, tag="sb", name=f"sb{l}_{w}"
                )
                nc.sync.dma_start(
                    out=sb[:, : nbw * 128], in_=S_d[:, o : o + int(pw[w])]
                )
                # SWDGE dma_gather tops out near 1024 descriptors per call on
                # HW (Q7 desc-ring batch limit) — split larger windows
                for c0 in range(0, int(pw[w]), GMAX):
                    cn = min(GMAX, int(pw[w]) - c0)
                    nc.gpsimd.dma_gather(
                        out_ap=gb[:, c0 // 128 : (c0 + cn) // 128, :],
                        in_ap=tables[l][:],
                        idxs_ap=idx_sb[:, (o + c0) // 16 : (o + c0 + cn) // 16],
                        num_idxs=cn,
                        num_idxs_reg=cn,
                        elem_size=128,
                    )
                acc = ps.tile([128, WIN], f32, tag="ps", name=f"agg{l}_{w}")
                for bb in range(nbw):
                    nc.tensor.matmul(
                        out=acc[:],
                        lhsT=gb[:, bb, :],
                        rhs=sb[:, bb * 128 : bb * 128 + WIN],
                        start=(bb == 0),
                        stop=(bb == nbw - 1),
                    )
                nc.scalar.activation(
                    out=aggT[:, w * WIN : w * WIN + wcols], in_=acc[:, :wcols],
                    func=AF.Copy,
                )

            # GRU per 512-column strip (Sigmoid/Tanh share one ACT table)
            for b, (s0, w) in enumerate(banks):
                gis, ghs = [], []
                for g in range(3):
                    gi_ps = ps.tile([128, BANK], f32, tag="ps", name=f"gi{l}{b}{g}")
                    nc.tensor.matmul(
                        out=gi_ps[:, :w], lhsT=bih_sb[:1, g * H : (g + 1) * H],
                        rhs=ones_row[:1, :w], start=True, stop=False,
                    )
                    nc.tensor.matmul(
                        out=gi_ps[:, :w], lhsT=wihT_sb[:, g * H : (g + 1) * H],
                        rhs=aggT[:, s0 : s0 + w], start=False, stop=True,
                    )
                    gi = gpool.tile([128, BANK], f32, tag=f"gi{g}")
                    nc.scalar.activation(out=gi[:, :w], in_=gi_ps[:, :w], func=AF.Copy)
                    gis.append(gi)
                    gh_ps = ps.tile([128, BANK], f32, tag="ps", name=f"gh{l}{b}{g}")
                    nc.tensor.matmul(
                        out=gh_ps[:, :w], lhsT=bhh_sb[:1, g * H : (g + 1) * H],
                        rhs=ones_row[:1, :w], start=True, stop=False,
                    )
                    nc.tensor.matmul(
                        out=gh_ps[:, :w], lhsT=whhT_sb[:, g * H : (g + 1) * H],
                        rhs=hT[:, s0 : s0 + w], start=False, stop=True,
                    )
                    gh = gpool.tile([128, BANK], f32, tag=f"gh{g}")
                    nc.scalar.activation(out=gh[:, :w], in_=gh_ps[:, :w], func=AF.Copy)
                    ghs.append(gh)

                # r = sig(i_r + h_r); z = sig(i_z + h_z); n = tanh(i_n + r*h_n)
                r_t = tpool.tile([128, BANK], f32, tag="r_t")
                nc.vector.tensor_tensor(
                    out=r_t[:, :w], in0=gis[0][:, :w], in1=ghs[0][:, :w], op=OP.add
                )
                nc.scalar.activation(out=r_t[:, :w], in_=r_t[:, :w], func=AF.Sigmoid)
                z_t = tpool.tile([128, BANK], f32, tag="z_t")
                nc.vector.tensor_tensor(
                    out=z_t[:, :w], in0=gis[1][:, :w], in1=ghs[1][:, :w], op=OP.add
                )
                nc.scalar.activation(out=z_t[:, :w], in_=z_t[:, :w], func=AF.Sigmoid)
                n_t = tpool.tile([128, BANK], f32, tag="n_t")
                nc.vector.tensor_tensor(
                    out=n_t[:, :w], in0=r_t[:, :w], in1=ghs[2][:, :w], op=OP.mult
                )
                nc.vector.tensor_tensor(
                    out=n_t[:, :w], in0=n_t[:, :w], in1=gis[2][:, :w], op=OP.add
                )
                nc.scalar.activation(out=n_t[:, :w], in_=n_t[:, :w], func=AF.Tanh)
                # h' = n + z * (h - n)
                d_t = tpool.tile([128, BANK], f32, tag="d_t")
                nc.vector.tensor_tensor(
                    out=d_t[:, :w], in0=hT[:, s0 : s0 + w], in1=n_t[:, :w],
                    op=OP.subtract,
                )
                nc.vector.tensor_tensor(
                    out=d_t[:, :w], in0=d_t[:, :w], in1=z_t[:, :w], op=OP.mult
                )
                nc.vector.tensor_tensor(
                    out=hT[:, s0 : s0 + w], in0=d_t[:, :w], in1=n_t[:, :w], op=OP.add
                )

            # batched l2norm + cross-layer max (one Sqrt table swap per layer)
            for b, (s0, w) in enumerate(banks):
                norm_strip(b, s0, w)
                nc.vector.tensor_tensor(
                    out=hmaxT[:, s0 : s0 + w], in0=hmaxT[:, s0 : s0 + w],
                    in1=hT[:, s0 : s0 + w], op=OP.max,
                )
            if l < L - 1:
                store_and_allgather(l + 1)

        # ---------------- decoder: out = hmax @ W2.T + b2 (row layout)
        for t in range(ntiles):
            wt = min(128, npc - t * 128)
            o_ps = ps.tile([128, 128], f32, tag="ps", name=f"dec{t}")
            nc.tensor.matmul(
                out=o_ps[:wt, :], lhsT=onesk1[:1, :wt], rhs=b2_sb[:1, :],
                start=True, stop=False,
            )
            nc.tensor.matmul(
                out=o_ps[:wt, :], lhsT=hmaxT[:, t * 128 : t * 128 + wt],
                rhs=w2T_sb[:], start=False, stop=True,
            )
            orow = tpool.tile([128, 128], f32, tag="orow")
            nc.scalar.activation(out=orow[:wt, :], in_=o_ps[:wt, :], func=AF.Copy)
            nc.sync.dma_start(
                out=out_d[t * 128 : t * 128 + wt, :], in_=orow[:wt, :]
            )
        stack.close()

    nc.compile()
    return nc


# ---------------------------------------------------------------- entry points
def _prep(inputs):
    x = np.asarray(inputs["x"], np.float32)
    edge_src = np.asarray(inputs["edge_src"], np.int64)
    edge_dst = np.asarray(inputs["edge_dst"], np.int64)
    n_nodes = x.shape[0]
    meta = _plan(edge_src, edge_dst, n_nodes)

    W1 = np.asarray(inputs["W1"], np.float32)
    b1 = np.asarray(inputs["b1"], np.float32)
    W_ih = np.asarray(inputs["W_ih"], np.float32)
    b_ih = np.asarray(inputs["b_ih"], np.float32)
    W_hh = np.asarray(inputs["W_hh"], np.float32)
    b_hh = np.asarray(inputs["b_hh"], np.float32)
    W2 = np.asarray(inputs["W2"], np.float32)
    b2 = np.asarray(inputs["b2"], np.float32)

    shared = dict(
        w1T=np.ascontiguousarray(W1.T),
        b1=np.ascontiguousarray(b1[:, None]),
        wihT=np.ascontiguousarray(W_ih.T),
        whhT=np.ascontiguousarray(W_hh.T),
        bih=np.ascontiguousarray(b_ih[None, :]),
        bhh=np.ascontiguousarray(b_hh[None, :]),
        w2T=np.ascontiguousarray(W2.T),
        b2=np.ascontiguousarray(b2[None, :]),
    )
    in_maps = []
    for r in range(CORES):
        xr = x[meta["order_per_core"][r]]
        in_maps.append(
            dict(
                xT=np.ascontiguousarray(xr.T),
                idxw=meta["idx_wrapped"][r],
                S=meta["S_wrapped"][r],
                **shared,
            )
        )
    return meta, in_maps


def _assemble(meta, results, n_nodes):
    npc = meta["npc"]
    out = np.empty((n_nodes, 128), np.float32)
    for r in range(CORES):
        out[meta["order_per_core"][r]] = results[r]["out"][:npc]
    return out


@functools.lru_cache(maxsize=1)
def _get_compiled(key):
    # key is a hash of the planning inputs; real data passed via _PENDING
    meta, in_maps = _PENDING[key]
    nc = _build(meta)
    return nc, meta, in_maps


_PENDING = {}


def kernel(**inputs):
    x = np.asarray(inputs["x"])
    n_nodes = x.shape[0]
    meta, in_maps = _prep(inputs)
    key = hash(
        (
            n_nodes,
            np.asarray(inputs["edge_src"]).tobytes(),
            np.asarray(inputs["edge_dst"]).tobytes(),
        )
    )
    _PENDING[key] = (meta, in_maps)
    nc, meta, _ = _get_compiled(key)

    from concourse.bass_utils import run_bass_kernel_spmd

    trace = bool(int(os.environ.get("KERNEL_TRACE", "0")))
    res = run_bass_kernel_spmd(
        nc, in_maps, core_ids=list(range(CORES)), trace=trace
    )
    kernel.last_results = res
    return _assemble(meta, res.results, n_nodes)
